# revision 88
# baseline (speedup 1.0000x reference)
"""Trainium2 Bass kernel: Longformer-style windowed attention with rotary,
head-averaged K/V (step_attn), fused QKV/out projections.

Sharding: 8 cores = (batch 2) x (sequence-quarter 4). Each core computes its
512 output rows for all 16 heads. No collectives: the windowed attention for
a 512-row quarter only needs 6 key-tiles (128 rows each) of the head-averaged
K/V plus the 64 global-token rows, all of which the core computes itself from
host-sliced hidden-state rows. Head-averaging of K/V commutes with rotary and
with the (linear) projection, so the K/V-mean projection weights are folded on
host to [2048, 256].

Key device-side structure (one SPMD program, per-core differences enter as
input data):
- hid arrives host-pre-transposed in row chunks; the kv projection consumes
  each chunk as it lands (bf16, full precision for the K/V path).
- q projection emits q TRANSPOSED directly (Wq as the stationary operand
  against hid^T) in fp8 DoubleRow (2x PE throughput; weights x64 on host to
  clear the fp8 subnormal range, undone in the evacuation activation, which
  also applies the per-partition q bias). Rotary touches only dims 0:32 =
  psum partitions 0:32 and is fixed up in place: a +-1 permutation matmul
  plus three elementwise ops against transposed bf16 trig tables.
- kT comes from PE identity-transpose matmuls, one per q-projection head.
- Attention runs a global software pipeline over (block, head-group)
  iterations: per iteration [po of i][out-proj quarter filler][scores of
  i+2]. Scores issue TWO iterations ahead so the exp/mask chain never
  stalls the PE; the aT normalize is issued on DVE before the next
  iteration's chain so the in-order DVE queue serves it first.
- softmax z: glob keys padded to 128 with a -30 exp bias, one z-sum tree +
  gpsimd partition_all_reduce (broadcasts z to all partitions in one op) +
  fast approximate reciprocal. The always-valid middle window tile needs no
  mask multiply; clipped-tile invalidity is folded into the exp bias.
- K-bias is dropped exactly (softmax shift invariance); V-bias and b_o fold
  into a single row added on HOST after gather; output returns as bf16 and
  is upcast on host.
"""

import sys

for _p in ("/opt/trn_rl_repo", "/root/.axon_site/_ro/trn_rl_repo"):
    if _p not in sys.path:
        sys.path.append(_p)

import numpy as np
import ml_dtypes

import concourse.bass as bass
import concourse.bass_isa as bass_isa
import concourse.tile as tile
from concourse import bacc
import concourse.mybir as mybir
from concourse.bass_utils import run_bass_kernel_spmd

F32 = mybir.dt.float32
BF16 = mybir.dt.bfloat16
FP8 = mybir.dt.float8e4
DR = mybir.MatmulPerfMode.DoubleRow
WSCALE = 64.0
MUL = mybir.AluOpType.mult
ADD = mybir.AluOpType.add
SUB = mybir.AluOpType.subtract

H = 16
D = 128
ROT = 32
HALF = 16  # ROT // 2
WIN = 256
G = 64
BASE = 10000.0
S = 2048
HD = H * D
B = 2
NCORES = 8
QROWS = 512          # rows per core
NKV = 6              # kv key-tiles per core
KVG_ROWS = NKV * 128 + G  # 832
SCALE = 1.0 / float(np.sqrt(np.float32(D)))

# pk128 column layout: ckv | skv | am | bg | bqT | cosqT | sinqT
PK_CKV = 0
PK_SKV = NKV * HALF                   # 96
PK_AM = 2 * NKV * HALF                # 192
PK_BG = PK_AM + NKV                   # 198
PK_BQ = PK_BG + 1                     # 199
PK_W = PK_BQ + H                      # 215


# ---------------------------------------------------------------- device ----

def build_nc():
    nc = bacc.Bacc("TRN2", target_bir_lowering=False, debug=False,
                   num_devices=NCORES)

    aps = {}
    def inp(name, shape, dt):
        aps[name] = nc.dram_tensor(name, shape, dt, kind="ExternalInput").ap()

    inp("hidT", [NKV + 1, 128, 16, 128], BF16)   # host pre-transposed, chunked
    inp("hidQ", [128, 16, QROWS], FP8)    # q rows again, row-contiguous
    inp("wq", [128, H, 16, D], FP8)       # [p, h, kt, d] = Wq[kt*128+p, h*D+d]
    inp("wkv", [HD, 2 * D], BF16)
    inp("wo", [HD, HD], BF16)
    inp("pk128", [128, PK_W], F32)
    inp("trigq", [ROT, 4 * QROWS], BF16)  # cos|sin q tables, transposed
    inp("pk64", [G, 2 * HALF], F32)
    inp("pkb", [1, 2 * HD + 2 * D], BF16)
    inp("p32", [ROT, ROT], BF16)          # rotate-half permutation (lhsT)
    inp("ident", [128, 128], BF16)        # identity for PE transposes
    inp("mask_win", [128, 4, 2, 128], BF16)
    inp("mask_glob", [G, QROWS], BF16)
    aps["out"] = nc.dram_tensor("out", [QROWS, HD], BF16,
                                kind="ExternalOutput").ap()

    with tile.TileContext(nc) as tc:
        _build_tile(nc, tc, aps)
    nc.compile()
    return nc


def _build_tile(nc, tc, aps):
    from contextlib import ExitStack
    import os
    ctx = ExitStack()
    _PH = int(os.environ.get("KERNEL_PHASES", "4"))

    persist = ctx.enter_context(tc.tile_pool(name="persist", bufs=1))
    dpool = ctx.enter_context(tc.tile_pool(name="dpool", bufs=2, space="DRAM"))
    # right-side pools released after the q projection
    ctxR = ExitStack()
    hidp = ctxR.enter_context(tc.tile_pool(name="hidp", bufs=1, side="right"))
    wpool = ctxR.enter_context(tc.tile_pool(name="wstream", bufs=7, side="right"))
    epool = ctxR.enter_context(tc.tile_pool(name="evac", bufs=3, side="right"))
    psE = ctxR.enter_context(tc.tile_pool(name="psE", bufs=1, space="PSUM", side="right"))

    # ---------------- persistent tiles
    hidT = hidp.tile([128, NKV + 1, 16, 128], BF16, tag="hidT")
    hidQ = hidp.tile([128, 16, QROWS], FP8, tag="hidQ")
    qT = persist.tile([128, H, QROWS], BF16, tag="qT")
    kv_sb = persist.tile([128, NKV, 2 * D], BF16, tag="kv_sb")
    kvg_sb = persist.tile([G, 2 * D], BF16, tag="kvg_sb")
    kT = persist.tile([128, NKV * 128], BF16, tag="kT")
    kgT = persist.tile([128, 128], BF16, tag="kgT")
    wkv_sb = persist.tile([128, 16, 2 * D], BF16, tag="wkv_sb")
    wo_sb = persist.tile([128, H, HD], BF16, tag="wo_sb")
    p32_sb = persist.tile([ROT, ROT], BF16, tag="p32")
    ident_sb = persist.tile([128, 128], BF16, tag="ident")
    pk128 = persist.tile([128, PK_W], F32, tag="pk128")
    pk64 = persist.tile([G, 2 * HALF], F32, tag="pk64")
    pkb = persist.tile([1, 2 * HD + 2 * D], BF16, tag="pkb")
    mw_sb = persist.tile([128, 4, 2, 128], BF16, tag="mw")
    mg_sb = persist.tile([G, QROWS], BF16, tag="mg")
    ckv_sb = pk128[:, PK_CKV:PK_SKV].rearrange("p (t r) -> p t r", r=HALF)
    skv_sb = pk128[:, PK_SKV:PK_AM].rearrange("p (t r) -> p t r", r=HALF)
    am_sb = pk128[:, PK_AM:PK_BG]
    bg_sb = pk128[:, PK_BG:PK_BG + 1]  # 0 for p<G, -30 for p>=G (glob pad)
    bqT_sb = pk128[:, PK_BQ:PK_BQ + H]
    trigq_sb = persist.tile([ROT, 4 * QROWS], BF16, tag="trigq")
    cqT_sb = trigq_sb[:, 0:QROWS]
    sqT_sb = trigq_sb[:, QROWS:2 * QROWS]
    cg_sb = pk64[:, 0:HALF]
    sg_sb = pk64[:, HALF:2 * HALF]
    btot_row = pkb[:, HD + 2 * D:2 * HD + 2 * D]   # b_o + tile(bv) @ W_o

    # ---------------- loads: wkv + hid transposes first (kv proj overlaps
    # the transposes), small tables after, wq streamed per-head in the q loop
    nc.gpsimd.memset(kgT[:], 0.0)
    # hid arrives host-pre-transposed in row chunks (one per kv tile);
    # kv-proj tile st is fully computable the moment chunk st lands
    nc.sync.dma_start(out=hidT[:, 0], in_=aps["hidT"][0])
    nc.sync.dma_start(out=wkv_sb[:],
                      in_=aps["wkv"].rearrange("(t p) c -> p t c", p=128))
    for st in range(1, NKV + 1):
        nc.sync.dma_start(out=hidT[:, st], in_=aps["hidT"][st])
    nc.sync.dma_start(out=hidQ[:], in_=aps["hidQ"])
    # wq streams issued up-front so the first q matmul never waits behind
    # later queue entries; pk tables slot in after wq0 (needed for kv rotary
    # and the first q evac)
    wq_tiles = []
    for h in range(H):
        wq_t = wpool.tile([128, 16, D], FP8, tag="wq_t", name=f"wq{h}")
        nc.sync.dma_start(out=wq_t[:], in_=aps["wq"][:, h])
        wq_tiles.append(wq_t)
        if h == 0:
            for nm, t in (("pk128", pk128), ("pk64", pk64), ("pkb", pkb),
                          ("p32", p32_sb), ("ident", ident_sb)):
                nc.sync.dma_start(out=t[:], in_=aps[nm])
        elif h == 5:
            for nm, t in (("trigq", trigq_sb), ("mask_win", mw_sb),
                          ("mask_glob", mg_sb)):
                nc.sync.dma_start(out=t[:], in_=aps[nm])

    # ---------------- kv projection (+ glob rows), tile-sequential; each
    # tile's rows arrive as one transpose chunk, double-buffered psum banks
    for st in range(NKV + 1):
        m = 128 if st < NKV else G
        pkv = psE.tile([128, 512], F32, tag="pq", bufs=2, name=f"pkv{st}")
        for kt in range(16):
            nc.tensor.matmul(pkv[:m, 0:2 * D],
                             hidT[:, st, kt, :m],
                             wkv_sb[:, kt, :], start=(kt == 0), stop=(kt == 15))
        if st < NKV:
            nc.scalar.copy(kv_sb[:, st, :], pkv[:, 0:2 * D])
        else:
            nc.scalar.copy(kvg_sb[:], pkv[:G, 0:2 * D])

    # rotary (in-place, f32 temps): x1' = x1*c - x2*s ; x2' = x2*c + x1*s
    def rotary(x1, x2, c, s, shape, tag):
        t1 = epool.tile(shape, F32, tag=tag + "1")
        t2 = epool.tile(shape, F32, tag=tag + "2")
        nc.vector.tensor_tensor(out=t1[:], in0=x1, in1=s, op=MUL)
        nc.vector.tensor_tensor(out=t2[:], in0=x2, in1=s, op=MUL)
        nc.vector.tensor_tensor(out=x1, in0=x1, in1=c, op=MUL)
        nc.vector.tensor_tensor(out=x1, in0=x1, in1=t2[:], op=SUB)
        nc.vector.tensor_tensor(out=x2, in0=x2, in1=c, op=MUL)
        nc.vector.tensor_tensor(out=x2, in0=x2, in1=t1[:], op=ADD)

    rotary(kv_sb[:, :, 0:HALF], kv_sb[:, :, HALF:2 * HALF],
           ckv_sb[:], skv_sb[:], [128, NKV, HALF], "rkv")
    rotary(kvg_sb[:, 0:HALF], kvg_sb[:, HALF:2 * HALF],
           cg_sb[:], sg_sb[:], [G, HALF], "rg")

    # kT via PE transposes (identity-permutation matmuls) -- no DRAM
    # roundtrip; issued inside the q loop (after head 2) so the PE never
    # waits on the kv rotary
    def kT_transpose_one(st):
        m = 128 if st < NKV else G
        ptr = psE.tile([128, 128], BF16, tag="ptr", bufs=1,
                       name=f"ptr{st}")
        src_k = kv_sb[:, st, 0:128] if st < NKV else kvg_sb[:, 0:128]
        nc.tensor.matmul(ptr[:, :m], src_k[:m], ident_sb[:m, :m],
                         is_transpose=True, start=True, stop=True)
        if st < NKV:
            nc.scalar.copy(kT[:, st * 128:(st + 1) * 128], ptr[:])
        else:
            nc.scalar.copy(kgT[:, 0:G], ptr[:, :G])


    if _PH < 2:
        ctxR.close()
        ctx.close()
        return

    # ---------------- attention pools + functions created BEFORE the q loop
    # so the first two score iterations can issue inside it. PSUM: "early"
    # ring (kv/q/perm, right) 4 banks + "ps" ring 4 banks coexist; the po2
    # pool takes the early ring's banks once it is released.
    ps = ctx.enter_context(tc.tile_pool(name="ps", bufs=4, space="PSUM"))
    wexp = ctx.enter_context(tc.tile_pool(name="wexp", bufs=8))
    rzp = ctx.enter_context(tc.tile_pool(name="rzp", bufs=4))
    opool = ctx.enter_context(tc.tile_pool(name="opool", bufs=1))
    aT_tiles = [None] * 4

    def attn_scores(L, hg):
        """Score matmuls with per-tile exp issued immediately after each
        matmul, masks only where needed (t=1 is always fully valid inside the
        window; clipped-tile invalidity is folded into the am exp-bias on
        host), then the z-sum tree + partition all-reduce + reciprocal.
        Returns (w_t, w_g, rz) for attn_po."""
        rhs_q = qT[:, 4 * hg:4 * hg + 4, L * 128:(L + 1) * 128]
        # glob first so its exp overlaps the window score matmuls on Act
        p_g = ps.tile([128, 512], F32, tag="ps", name=f"p_g{L}_{hg}")
        nc.tensor.matmul(p_g[:], kgT[:], rhs_q, start=True, stop=True)
        w_g = wexp.tile([128, 512], BF16, tag="wexpg", bufs=4,
                        name=f"wg{L}_{hg}")
        nc.scalar.activation(w_g[:], p_g[:],
                             mybir.ActivationFunctionType.Exp,
                             bias=bg_sb[:, 0:1], scale=SCALE)
        w_t = []
        for t in range(3):
            p_t = ps.tile([128, 512], F32, tag="ps", name=f"p_t{L}_{hg}_{t}")
            nc.tensor.matmul(p_t[:], kT[:, (L + t) * 128:(L + t + 1) * 128],
                             rhs_q, start=True, stop=True)
            w = wexp.tile([128, 512], BF16, tag="wexp", bufs=9, name=f"w{L}_{hg}_{t}")
            nc.scalar.activation(w[:], p_t[:],
                                 mybir.ActivationFunctionType.Exp,
                                 bias=am_sb[:, L + t:L + t + 1], scale=SCALE)
            w_t.append(w)
        nc.vector.tensor_tensor(
            out=w_g[:G].rearrange("g (h s) -> g h s", s=128),
            in0=w_g[:G].rearrange("g (h s) -> g h s", s=128),
            in1=mg_sb[:, L * 128:(L + 1) * 128]
                .rearrange("g (o s) -> g o s", o=1)
                .to_broadcast([G, 4, 128]),
            op=MUL)
        for sl, t in enumerate((0, 2)):
            nc.vector.tensor_tensor(
                out=w_t[t][:].rearrange("p (h s) -> p h s", s=128),
                in0=w_t[t][:].rearrange("p (h s) -> p h s", s=128),
                in1=mw_sb[:, L, sl:sl + 1, :].to_broadcast([128, 4, 128]),
                op=MUL)

        # z-sum tree on two engines, then one partition all-reduce (gives z
        # broadcast to every partition), reciprocal off the PE critical path
        s01 = rzp.tile([128, 512], BF16, tag="s01", bufs=2, name=f"s01_{L}_{hg}")
        s2g = rzp.tile([128, 512], BF16, tag="s2g", bufs=2, name=f"s2g_{L}_{hg}")
        nc.vector.tensor_tensor(out=s01[:], in0=w_t[0][:], in1=w_t[1][:],
                                op=ADD)
        nc.vector.tensor_tensor(out=s2g[:], in0=w_t[2][:], in1=w_g[:], op=ADD)
        tsum = rzp.tile([128, 512], BF16, tag="tsum", bufs=2, name=f"ts{L}_{hg}")
        nc.vector.tensor_tensor(out=tsum[:], in0=s01[:], in1=s2g[:], op=ADD)
        zz = rzp.tile([128, 512], F32, tag="zz", bufs=2, name=f"zz{L}_{hg}")
        nc.gpsimd.partition_all_reduce(zz[:], tsum[:], channels=128,
                                       reduce_op=bass_isa.ReduceOp.add)
        rz = rzp.tile([128, 512], F32, tag="rz", bufs=3, name=f"rz{L}_{hg}")
        nc.vector.reciprocal_approx_fast(out=rz[:], in_=zz[:])
        return w_t, w_g, rz

    def attn_po(L, hg, aT, w_t, w_g, rz):
        po = ps.tile([128, 512], F32, tag="ps", name=f"po{L}_{hg}")
        for t in range(3):
            nc.tensor.matmul(po[:], kv_sb[:, L + t, D:2 * D], w_t[t][:],
                             start=(t == 0), stop=False)
        nc.tensor.matmul(po[:], kvg_sb[:, D:2 * D], w_g[:G],
                         start=False, stop=True)
        nc.vector.tensor_tensor(
            out=aT[:, 4 * hg:4 * hg + 4, :],
            in0=po[:].rearrange("p (h s) -> p h s", s=128),
            in1=rz[:].rearrange("p (h s) -> p h s", s=128),
            op=MUL)

    # out-projection in head-quarters: quarter hq of block Lp accumulates
    # heads 4hq..4hq+3 into four [128,512] psum tiles (all 2048 out cols);
    # quarter 3 evacuates (+bias) and writes out. Quarter hq is issued as
    # the PE filler inside attention iteration (L, hg) per the schedule in
    # the main loop, which keeps at most one quarter-set (4 banks) live.
    oproj_state = {}

    def oproj_q(Lp, hq):
        aT = aT_tiles[Lp]
        if hq == 0:
            oproj_state[Lp] = [po2p.tile([128, 512], F32, tag="po2", bufs=4,
                                         name=f"po2_{Lp}_{i}") for i in range(4)]
        po2 = oproj_state[Lp]
        if hq < 3:
            for h in range(4 * hq, 4 * hq + 4):
                for ncn in range(4):
                    nc.tensor.matmul(po2[ncn][:], aT[:, h, :],
                                     wo_sb[:, h, ncn * 512:(ncn + 1) * 512],
                                     start=(h == 0), stop=False)
        else:
            # last quarter: ncn-outer so each column block evacuates (+bias)
            # and writes out while the next block's matmuls still run; the

            o_sb = opool.tile([128, HD], BF16, tag="o_sb", name=f"o_sb{Lp}")
            for ncn in range(4):
                for h in range(12, 16):
                    nc.tensor.matmul(po2[ncn][:], aT[:, h, :],
                                     wo_sb[:, h, ncn * 512:(ncn + 1) * 512],
                                     start=False, stop=(h == H - 1))
                nc.scalar.copy(o_sb[:, ncn * 512:(ncn + 1) * 512],
                               po2[ncn][:])
                nc.sync.dma_start(
                    out=aps["out"][Lp * 128:(Lp + 1) * 128,
                                   ncn * 512:(ncn + 1) * 512],
                    in_=o_sb[:, ncn * 512:(ncn + 1) * 512])
    # Global software pipeline over iterations i = (L, hg). PE issue order
    # per i: [oproj quarter filler][po of i][scores of i+1]. The po matmuls
    # of i only need exp+mask of i (ready: the filler covers that latency);
    # crucially the aT normalize (DVE) for i is issued BEFORE iteration
    # i+1's mask/z/recip DVE ops, so the in-order DVE queue serves it first
    # and the next filler (which reads aT) never waits on a whole chain.
    iters = [(L, hg) for L in range(4) for hg in range(4)]
    for L in range(4):
        aT_tiles[L] = wexp.tile([128, H, 128], BF16, tag="aT", bufs=2,
                                name=f"aT{L}")

    # ---------------- q projection, directly transposed: qT[:, h, :] =
    # (Wq_h)^T @ hid^T (fp8 DoubleRow, weights x64 undone in the evac),
    # bias via per-partition Identity-activation, rotary fixed up in place
    # (perm matmul deferred one head to hide the evac latency). The first
    # two score iterations issue inside the loop tail so their chains
    # complete by the time the main pipeline starts.
    def q_fixup(h):
        pp = psE.tile([128, 512], F32, tag="pp", bufs=1, name=f"pp{h}")
        nc.tensor.matmul(pp[0:ROT, :], p32_sb[:], qT[0:ROT, h, :],
                         start=True, stop=True)
        tmp = epool.tile([ROT, 512], BF16, tag="rqs", bufs=2, name=f"tq{h}")
        nc.vector.tensor_tensor(out=tmp[:], in0=pp[0:ROT, :], in1=sqT_sb,
                                op=MUL)
        nc.gpsimd.tensor_tensor(out=qT[0:ROT, h, :], in0=qT[0:ROT, h, :],
                                in1=cqT_sb, op=MUL)
        nc.vector.tensor_tensor(out=qT[0:ROT, h, :], in0=qT[0:ROT, h, :],
                                in1=tmp[:], op=ADD)

    # scores run TWO iterations ahead: ~8.5us of PE work separates scores(i)
    # from po(i), fully hiding the exp/mask chain; the 4-buf psum ring still
    # rotates cleanly because each tile's readers finish within one lap
    w_states = {}
    for h in range(H):
        wq_t = wq_tiles[h]
        pq = psE.tile([128, 512], F32, tag=("pp" if h == 0 else "pq"),
                      bufs=(1 if h == 0 else 2), name=f"pq{h}")
        for j in range(8):
            nc.tensor.matmul(pq[:], wq_t[:, 2 * j:2 * j + 2, :],
                             hidQ[:, 2 * j:2 * j + 2, :],
                             start=(j == 0), stop=(j == 7), perf_mode=DR)
        nc.scalar.activation(qT[:, h, :], pq[:],
                             mybir.ActivationFunctionType.Identity,
                             bias=bqT_sb[:, h:h + 1], scale=1.0 / WSCALE)
        if h > 0:
            q_fixup(h - 1)
        if 2 <= h <= 2 + NKV:
            kT_transpose_one(h - 2)
    q_fixup(H - 1)
    w_states[(0, 0)] = attn_scores(0, 0)
    w_states[(0, 1)] = attn_scores(0, 1)

    for i in range(4):
        nc.sync.dma_start(
            out=wo_sb[:, 4 * i:4 * (i + 1), :],
            in_=aps["wo"].rearrange("(h p) n -> p h n", p=128)[:, 4 * i:4 * (i + 1), :])

    ctxR.close()
    po2p = ctx.enter_context(tc.tile_pool(name="po2p", bufs=1, space="PSUM"))
    if _PH < 3:
        ctx.close()
        return
    for idx, (L, hg) in enumerate(iters):
        attn_po(L, hg, aT_tiles[L], *w_states.pop((L, hg)))
        if _PH >= 4:
            if hg == 0 and L >= 1:
                oproj_q(L - 1, 3)
            elif hg > 0:
                oproj_q(L, hg - 1)
        if idx + 2 < len(iters):
            nxt = iters[idx + 2]
            w_states[nxt] = attn_scores(*nxt)
    if _PH >= 4:
        oproj_q(3, 3)

    ctx.close()


# ------------------------------------------------------------------ host ----

_NC_CACHE = None


def _get_nc():
    global _NC_CACHE
    if _NC_CACHE is None:
        _NC_CACHE = build_nc()
    return _NC_CACHE


def make_in_maps(hidden_states, attention_mask, glob_idx, W_qkv, b_qkv, W_o, b_o):
    bf = ml_dtypes.bfloat16
    hidden_states = np.asarray(hidden_states, np.float32)
    attention_mask = np.asarray(attention_mask, np.float32)
    glob_idx = np.asarray(glob_idx)
    W_qkv = np.asarray(W_qkv, np.float32)
    b_qkv = np.asarray(b_qkv, np.float32)
    W_o = np.asarray(W_o, np.float32)
    b_o = np.asarray(b_o, np.float32)

    w3 = W_qkv.reshape(HD, H, 3 * D)
    f8 = ml_dtypes.float8_e4m3
    # wq as stationary-operand layout [p, h, kt, d] = Wq[kt*128+p, h*D+d];
    # x64 moves fp8 values out of the subnormal range (undone in the evac)
    wq = np.ascontiguousarray(
        w3[:, :, :D].reshape(16, 128, H, D).transpose(1, 2, 0, 3)
        * WSCALE).astype(f8)
    wkv = np.concatenate([w3[:, :, D:2 * D].mean(axis=1),
                          w3[:, :, 2 * D:].mean(axis=1)], axis=1).astype(bf)
    b3 = b_qkv.reshape(H, 3 * D)
    bq = np.ascontiguousarray(b3[:, :D].reshape(1, HD)).astype(bf)
    bkv = np.concatenate([b3[:, D:2 * D].mean(axis=0),
                          b3[:, 2 * D:].mean(axis=0)])[None, :].astype(bf)
    # K-bias is dropped (softmax is invariant to a per-query constant shift);
    # V-bias contributes tile(bv, H) @ W_o to every output row -> fold w/ b_o.
    bv_mean = b3[:, 2 * D:].mean(axis=0)                      # [D]
    b_total = (b_o + np.tile(bv_mean, H).astype(np.float32) @ W_o)[None, :]
    bo = b_total.astype(bf)
    wo = W_o.astype(bf)

    inv_freq = 1.0 / (BASE ** (np.arange(0, ROT, 2, dtype=np.float32) / ROT))
    freqs = np.arange(S, dtype=np.float32)[:, None] * inv_freq[None, :]  # [S,16]
    cos_all = np.cos(freqs).astype(np.float32)
    sin_all = np.sin(freqs).astype(np.float32)

    b3f = b_qkv.reshape(H, 3 * D)
    bqT = b3f[:, :D].T.astype(np.float32)                 # [D, H]

    p32 = np.zeros((ROT, ROT), np.float32)                # lhsT of rotate-half
    for j in range(HALF):
        p32[j + HALF, j] = -1.0
        p32[j, j + HALF] = 1.0
    p32 = p32.astype(bf)

    in_maps = []
    for c in range(NCORES):
        b, q = divmod(c, 4)
        t0 = 4 * q - 2
        tiles = [max(0, t0 + i) for i in range(NKV)]       # clipped content
        intended = [t0 + i for i in range(NKV)]
        kv_rows = np.concatenate([np.arange(t * 128, t * 128 + 128)
                                  for t in tiles])
        g_rows = glob_idx[b].astype(np.int64)
        rows = np.concatenate([kv_rows, g_rows])
        hid_c = hidden_states[b][rows]
        # pre-transpose to [chunk, p, kt, r]: hidT[st, p, kt, r] =
        # hid[st*128+r, kt*128+p]; glob chunk padded to 128 rows
        hid_pad = np.zeros(((NKV + 1) * 128, HD), np.float32)
        hid_pad[:KVG_ROWS] = hid_c
        hidT_host = np.ascontiguousarray(
            hid_pad.reshape(NKV + 1, 128, 16, 128).transpose(0, 3, 2, 1)
        ).astype(bf)
        hidQ_host = np.ascontiguousarray(
            hid_pad[256:256 + QROWS].reshape(QROWS, 16, 128)
            .transpose(2, 1, 0)).astype(f8)

        q_rows = np.arange(QROWS * q, QROWS * (q + 1))
        # transposed rotary tables for the core's 512 q rows: [ROT, 512]
        # (dim j uses inv_freq[j % HALF]), bf16, padded to 4*QROWS cols
        trigq = np.zeros((ROT, 4 * QROWS), np.float32)
        trigq[:, 0:QROWS] = np.tile(cos_all[q_rows].T, (2, 1))
        trigq[:, QROWS:2 * QROWS] = np.tile(sin_all[q_rows].T, (2, 1))
        cos_kv = cos_all[kv_rows].reshape(NKV, 128, HALF).transpose(1, 0, 2).copy()
        sin_kv = sin_all[kv_rows].reshape(NKV, 128, HALF).transpose(1, 0, 2).copy()
        cos_g = cos_all[g_rows].copy()
        sin_g = sin_all[g_rows].copy()

        am = attention_mask[b, 0, 0]                        # [S]
        am_loc = am[kv_rows].reshape(NKV, 128).T.copy()     # [128, NKV]
        # clipped (non-existent) kv tiles: kill the whole slot via exp bias
        # so the always-valid middle window tile needs no mask multiply
        for sl in range(NKV):
            if t0 + sl < 0:
                am_loc[:, sl] += -30.0

        # window masks [128 key-p, 4 L, 3 t, 128 s]: valid iff
        # row-(WIN-1) <= key_pos <= row and the slot's intended tile exists
        mask_win = np.zeros((128, 4, 3, 128), np.float32)
        for L in range(4):
            rows_glb = QROWS * q + L * 128 + np.arange(128)          # [s]
            for t in range(3):
                it = intended[L + t]
                if it < 0:
                    continue
                key_pos = it * 128 + np.arange(128)                  # [p]
                valid = (key_pos[:, None] <= rows_glb[None, :]) & \
                        (key_pos[:, None] >= rows_glb[None, :] - (WIN - 1))
                mask_win[:, L, t, :] = valid
        # glob mask [64, 512]: row >= WIN and glob_idx < row - WIN
        rows_glb = QROWS * q + np.arange(QROWS)
        mask_glob = ((rows_glb[None, :] >= WIN) &
                     (g_rows[:, None] < rows_glb[None, :] - WIN)).astype(np.float32)

        bg_col = np.where(np.arange(128) < G, 0.0, -30.0)[:, None]
        bqT_pad = np.zeros((128, H), np.float32)
        bqT_pad[:D] = bqT
        pk128 = np.concatenate(
            [cos_kv.reshape(128, 96), sin_kv.reshape(128, 96),
             am_loc, bg_col, bqT_pad], axis=1).astype(np.float32)
        pk64 = np.concatenate([cos_g, sin_g], axis=1).astype(np.float32)
        pkb = np.concatenate([bq, bkv, bo], axis=1)
        in_maps.append({
            "hidT": hidT_host, "hidQ": hidQ_host,
            "ident": np.eye(128, dtype=np.float32).astype(bf),
            "trigq": trigq.astype(bf),
            "wq": wq, "wkv": wkv, "wo": wo,
            "pk128": pk128, "pk64": pk64, "pkb": pkb, "p32": p32,
            "mask_win": np.ascontiguousarray(mask_win[:, :, (0, 2), :]).astype(bf),
            "mask_glob": mask_glob.astype(bf),
        })
    return in_maps


def kernel(hidden_states, attention_mask, glob_idx, W_qkv, b_qkv, W_o, b_o):
    global _BTOT
    b3h = np.asarray(b_qkv, np.float32).reshape(H, 3 * D)
    _BTOT = (np.asarray(b_o, np.float32)
             + np.tile(b3h[:, 2 * D:].mean(axis=0), H)
             @ np.asarray(W_o, np.float32))[None, :]
    nc = _get_nc()
    in_maps = make_in_maps(hidden_states, attention_mask, glob_idx,
                           W_qkv, b_qkv, W_o, b_o)
    res = run_bass_kernel_spmd(nc, in_maps, core_ids=list(range(NCORES)))
    out = np.empty((B, S, HD), np.float32)
    for c in range(NCORES):
        b, q = divmod(c, 4)
        out[b, QROWS * q:QROWS * (q + 1), :] = np.asarray(
            res.results[c]["out"], np.float32) + _BTOT
    return out

